# revision 22
# baseline (speedup 1.0000x reference)
"""Trainium2 Bass kernel for nn_CapsuleLayer (dynamic routing).

Problem:  u_hat = einsum('bri,crio->cbro', x, W);  3 routing iterations
          (softmax over R, weighted sum, squash, agreement update).
Shapes:   x [256, 1152, 8] f32, W [10, 1152, 8, 16] f32 ->
          out [10, 256, 1, 1, 16] f32.

Strategy (8 NeuronCores, data-parallel over batch, B_loc = 32/core):
  * never materialize u_hat (189 MB) in HBM; all PE data fp16; every
    partition-sliced access sits on a 32-row boundary (walrus rule).
  * s-sums   : per K-block three 128-col stationaries (4+4+2 classes,
    one class per 32-col group: 16 real + 16 pad cols) stream the
    class-blocked y (diagonal blocks used), FWL weight loads; y = cw*x
    built fp16 on DVE/GPSIMD in two q-half stages.
  * agreement: classes stacked into K=128 — per-wave block-diagonal v
    stationary [128,128] (rows 32j+o hold class 4w+j, dead rows zero)
    against an o-major W copy, N=512 streams; ACT drains PSUM->SBUF
    fp16, DVE multiplies by x (2x mode), i-tree l1 on GPSIMD, rest on
    DVE; L fp32.
  * softmax  : ACT exp fp32 (overflow-safe) with accumulated Z, fp16
    normalize, single-pass fp16 PE transposes to r-block partitions.
"""

import sys
from contextlib import ExitStack

import numpy as np

sys.path.insert(0, "/opt/trn_rl_repo")

import concourse.bacc as bacc
import concourse.bass as bass
import concourse.mybir as mybir
import concourse.tile as tile
from concourse.bass_utils import run_bass_kernel_spmd

F32 = mybir.dt.float32
F16 = mybir.dt.float16
MUL = mybir.AluOpType.mult
ADD = mybir.AluOpType.add

B, R, I, C, O = 256, 1152, 8, 10, 16
NC = 8
BL = B // NC          # 32 batch per core
Q = R // 128          # 9 r-blocks of 128
RI = R * I            # 9216
EPS = 1e-7
GCH = 1024            # agreement free-dim chunk (elements of (r,i))
NG = RI // GCH        # 9 chunks
W3 = 3                # (c,b) waves
CB = C * BL           # 320
WF = 320              # wfr columns: 10 class-slots x 32 (16 real + 16 pad)


def build_nc(debug=False):
    nc = bacc.Bacc("TRN2", target_bir_lowering=False, debug=debug)

    xtr_d = nc.declare_dram_parameter("xtr", [128, Q, I, BL], F16, isOutput=False)
    wfr_d = nc.declare_dram_parameter("wfr", [128, Q, I, WF], F16, isOutput=False)
    wte_d = nc.declare_dram_parameter("wte", [2, 128, RI], F16, isOutput=False)
    wt2_d = nc.declare_dram_parameter("wt2", [128, RI // 2], F16, isOutput=False)
    xrep_d = nc.declare_dram_parameter("xrep", [128, RI], F16, isOutput=False)
    ident_d = nc.declare_dram_parameter("ident", [128, 128], F16, isOutput=False)
    out_d = nc.declare_dram_parameter("out", [C, O, BL], F32, isOutput=True)

    with tile.TileContext(nc) as tc, ExitStack() as ctx:
        res = ctx.enter_context(tc.tile_pool(name="res", bufs=1))
        cwp = ctx.enter_context(tc.tile_pool(name="cwp", bufs=1))
        yp = ctx.enter_context(tc.tile_pool(name="yp", bufs=2))
        gsp = ctx.enter_context(tc.tile_pool(name="gsp", bufs=2))
        gmp = ctx.enter_context(tc.tile_pool(name="gmp", bufs=1))
        trp = ctx.enter_context(tc.tile_pool(name="trp", bufs=2))
        smp = ctx.enter_context(tc.tile_pool(name="smp", bufs=1))
        psS = ctx.enter_context(
            tc.tile_pool(name="psS", bufs=1, space=bass.MemorySpace.PSUM)
        )
        psG = ctx.enter_context(
            tc.tile_pool(name="psG", bufs=2, space=bass.MemorySpace.PSUM)
        )

        # ---- resident tensors -------------------------------------
        xtr = res.tile([128, Q, I, BL], F16)
        wfr = res.tile([128, Q, I, WF], F16)
        wte = res.tile([128, 2, RI], F16)    # rows 32j+o = W[4w+j]
        wt2 = res.tile([128, RI // 2], F16)  # wave-2 plane folded in half
        xrep = res.tile([128, RI], F16)
        ident = res.tile([128, 128], F16)
        L = res.tile([128, W3, R], F16)
        cwT = res.tile([128, Q, 2, 128], F16)
        cwT2 = res.tile([128, Q, 64], F16)
        Z = res.tile([128, W3], F32)
        Zi = res.tile([128, W3], F32)
        vsE = res.tile([128, W3, 128], F16)  # block-diag v per wave
        ones16 = res.tile([16, 16], F32)
        sps_sb = res.tile([16, C, BL], F32)  # extracted s, [o, c, b]
        v_sb = res.tile([16, C, BL], F32)    # squash output, [o, c, b]

        nc.sync.dma_start(xtr[:], xtr_d[:])
        for q in range(3):
            nc.sync.dma_start(wfr[:, q], wfr_d[:, q])
        nc.sync.dma_start(wte[:, 0, :], wte_d[0])
        for q in range(3, Q):
            nc.sync.dma_start(wfr[:, q], wfr_d[:, q])
        nc.sync.dma_start(xrep[:], xrep_d[:])
        nc.sync.dma_start(wte[:, 1, :], wte_d[1])
        nc.sync.dma_start(wt2[:], wt2_d[:])
        nc.sync.dma_start(ident[:], ident_d[:])
        nc.vector.memset(ones16[:], 1.0)
        nc.vector.memset(vsE[:], 0.0)

        # class c -> (s_pass group g, 32-row/col slot j)
        def slot(c):
            return (c // 4, c % 4) if c < 8 else (2, c - 8)

        # ---------------------------------------------------------------
        def build_y(c, qlo, qhi, yall):
            """y[rr, q, i, 128g+32j+b] = cw[c,b,128q+rr] * x[b,128q+rr,i]."""
            w = min(c // 4, 2)
            k = c - 4 * w
            g, j = slot(c)
            nq = qhi - qlo
            cw_blk = (
                cwT[:, qlo:qhi, w, 32 * k : 32 * k + 32]
                if w < 2
                else cwT2[:, qlo:qhi, 32 * k : 32 * k + 32]
            )
            cw_src = cw_blk.unsqueeze(2).broadcast_to([128, nq, I, BL])
            col = 128 * g + 32 * j
            eng = nc.gpsimd if c in (3, 6, 9) else nc.vector
            eng.tensor_tensor(
                yall[:, 0 : qhi - qlo, :, col : col + 32],
                xtr[:, qlo:qhi],
                cw_src,
                MUL,
            )

        def s_pass(it):
            """s[o,c,b] = sum_{r,i} rhs_c[r,i,b] * W[c,r,i,o].

            Three stationaries per K-block (classes 0-3 / 4-7 / 8-9, one
            class per 32-col group, pad cols zero).  it==0 streams xtr
            (shared rhs); it>0 streams the class-blocked y (diagonal
            class blocks used)."""
            ps = [
                psS.tile([128, 128 if it else BL], F32, tag="sps0", name="sps0"),
                psS.tile([128, 128 if it else BL], F32, tag="sps1", name="sps1"),
                psS.tile([128, 64 if it else BL], F32, tag="spst", name="spst"),
            ]
            nn = [128, 128, 64] if it else [BL, BL, BL]
            g = 0
            for qlo, qhi in ((0, 5), (5, 9)):
                yall = None
                if it:
                    yall = yp.tile(
                        [128, 5, I, 320], F16, tag="yall", name="yall"
                    )
                    for c in range(C):
                        build_y(c, qlo, qhi, yall)
                for q in range(qlo, qhi):
                    for i in range(I):
                        for grp in range(3):
                            if it:
                                base = 128 * grp
                                rhs = yall[:, q - qlo, i, base : base + nn[grp]]
                            else:
                                rhs = xtr[:, q, i, :]
                            m = 128 if grp < 2 else 64
                            nc.tensor.matmul(
                                ps[grp][0:m, :],
                                wfr[:, q, i, 128 * grp : 128 * grp + m],
                                rhs,
                                start=(g == 0),
                                stop=(g == Q * I - 1),
                            )
                        g += 1
            # extract per-class [16,32] diagonal blocks into [o, c, b]
            for c in range(C):
                grp, j = slot(c)
                cols = slice(32 * j, 32 * j + 32) if it else slice(0, BL)
                nc.vector.tensor_copy(
                    sps_sb[:, c, :], ps[grp][32 * j : 32 * j + 16, cols]
                )

        # ---------------------------------------------------------------
        def squash(it):
            """v_sb = squash(s) over o;  it==0 folds the uniform 1/R weight.

            Scalars kept [16, C*BL] (replicated rows via the ones16 matmul)
            so no partition broadcast is needed."""
            sq = smp.tile([16, C * BL], F32, tag="sq")
            nc.scalar.activation(
                sq[:],
                sps_sb[:].rearrange("o c b -> o (c b)"),
                mybir.ActivationFunctionType.Square,
            )
            snps = psS.tile([16, C * BL], F32, tag="spst")
            nc.tensor.matmul(snps[:], ones16[:], sq[:], start=True, stop=True)
            sn = smp.tile([16, C * BL], F32, tag="sn")
            if it == 0:
                nc.vector.tensor_scalar_mul(sn[:], snps[:], 1.0 / (R * R))
            else:
                nc.vector.tensor_copy(sn[:], snps[:])
            u1 = smp.tile([16, C * BL], F32, tag="u1")
            u2 = smp.tile([16, C * BL], F32, tag="u2")
            u3 = smp.tile([16, C * BL], F32, tag="sq")   # sq slot is free now
            u4 = smp.tile([16, C * BL], F32, tag="u1")   # u1 free after sqrt
            ub = smp.tile([16, C * BL], F32, tag="u2")   # u2 free after u4
            f = smp.tile([16, C * BL], F32, tag="sq")    # u3 free after u4
            nc.vector.tensor_scalar_add(u1[:], sn[:], EPS)
            nc.scalar.activation(u2[:], u1[:], mybir.ActivationFunctionType.Sqrt)
            nc.vector.tensor_scalar_add(u3[:], sn[:], 1.0)
            nc.vector.tensor_tensor(u4[:], u2[:], u3[:], MUL)
            nc.vector.reciprocal(ub[:], u4[:])
            nc.vector.tensor_tensor(f[:], sn[:], ub[:], MUL)
            if it == 0:
                nc.vector.tensor_scalar_mul(f[:], f[:], 1.0 / R)
            nc.vector.tensor_tensor(
                v_sb[:].rearrange("o c b -> o (c b)"),
                sps_sb[:].rearrange("o c b -> o (c b)"),
                f[:],
                MUL,
            )

        def fill_vs():
            """Stage v into the per-wave block-diagonal stationaries."""
            for c in range(C):
                w = min(c // 4, 2)
                j = c - 4 * w
                eng = nc.vector if c % 2 == 0 else nc.gpsimd
                eng.tensor_copy(
                    vsE[32 * j : 32 * j + 16, w, 32 * j : 32 * j + 32],
                    v_sb[:, c, :],
                )
                if c >= 8:  # duplicate for the folded wave-2 W plane
                    eng.tensor_copy(
                        vsE[64 + 32 * j : 64 + 32 * j + 16, w,
                            32 * j : 32 * j + 32],
                        v_sb[:, c, :],
                    )

        # ---------------------------------------------------------------
        def agreement(it):
            """L[p, w, r] (+)= sum_o v[c,b,o]*W[c,r,i,o] (*) x[b,r,i], sum_i.

            One K=128 matmul per 512-chunk per wave (block-diagonal v
            against o-major W); chunk pipeline: PE -> ACT drain fp16 ->
            DVE multiply -> GPSIMD l1 -> DVE l2/l3 -> L fp32."""
            nr = GCH // I  # 128 r per chunk
            for n0 in range(NG):
                off = n0 * GCH
                r0 = off // I
                # gm[p, w, (r i)] for all three waves of this chunk
                gm = gmp.tile([128, W3, GCH], F16, tag="gm")
                H = RI // 2
                for w in range(W3):
                    npart = 128 if w < 2 else 64
                    gps = psG.tile([128, GCH], F32, tag="gps")
                    if w < 2:
                        for sub in range(0, GCH, 512):
                            nc.tensor.matmul(
                                gps[:, sub : sub + 512],
                                vsE[:, w, :],
                                wte[:, w, off + sub : off + sub + 512],
                                start=True,
                                stop=True,
                            )
                    else:
                        # folded plane: pick the 64-row half per (ri) range
                        o0 = off
                        while o0 < off + GCH:
                            half = o0 // H
                            o1 = min(off + GCH, (half + 1) * H, o0 + 512)
                            nc.tensor.matmul(
                                gps[:, o0 - off : o1 - off],
                                vsE[64 * half : 64 * half + 64, 2, :],
                                wt2[64 * half : 64 * half + 64,
                                    o0 - half * H : o1 - half * H],
                                start=True,
                                stop=True,
                            )
                            o0 = o1
                    # ACT drain PSUM -> SBUF fp16 (frees PSUM, DVE gets 2x)
                    gs = gsp.tile([128, GCH], F16, tag="gs")
                    nc.scalar.copy(gs[:npart, :], gps[:npart, :])
                    # gm_w = gs * xrep  (fp16, contiguous); w2 on GPSIMD
                    eng = nc.vector if w < 2 else nc.gpsimd
                    eng.tensor_tensor(
                        gm[:npart, w, :],
                        gs[:npart, :],
                        xrep[:npart, off : off + GCH],
                        MUL,
                    )
                # i-tree batched over waves (8 -> 4 -> 2 -> L);
                # l2/l3 run in place inside l1's tile
                l1 = trp.tile([128, W3, GCH // 2], F16, tag="l1")
                gmv = gm.rearrange("p w (r i) -> p w r i", i=I)
                l1v = l1.rearrange("p w (r i) -> p w r i", i=4)
                nc.vector.tensor_tensor(
                    l1v[:], gmv[:, :, :, 0:4], gmv[:, :, :, 4:8], ADD
                )
                nc.gpsimd.tensor_tensor(
                    l1v[:, :, :, 0:2], l1v[:, :, :, 0:2], l1v[:, :, :, 2:4],
                    ADD,
                )
                if it == 0:
                    nc.vector.tensor_tensor(
                        L[:, :, r0 : r0 + nr],
                        l1v[:, :, :, 0],
                        l1v[:, :, :, 1],
                        ADD,
                    )
                else:
                    nc.gpsimd.tensor_tensor(
                        l1v[:, :, :, 0], l1v[:, :, :, 0], l1v[:, :, :, 1], ADD
                    )
                    nc.vector.tensor_tensor(
                        L[:, :, r0 : r0 + nr],
                        L[:, :, r0 : r0 + nr],
                        l1v[:, :, :, 0],
                        ADD,
                    )

        # ---------------------------------------------------------------
        def softmax_transpose():
            """cw = softmax_r(L); exp fp32 (overflow-safe), fp16 normalize,
            single-pass fp16 transposes into cwT."""
            T3 = R // 3
            for w in range(W3):
                cwn = cwp.tile([128, R], F16, tag="cwn")
                Zp = cwp.tile([128, 3], F32, tag="Zp")
                cwvs = []
                for t in range(3):
                    cwv = cwp.tile([128, T3], F32, tag=f"cwv{t}", name="cwv")
                    nc.scalar.activation(
                        cwv[:],
                        L[:, w, T3 * t : T3 * t + T3],
                        mybir.ActivationFunctionType.Exp,
                        accum_out=Zp[:, t : t + 1],
                    )
                    cwvs.append(cwv)
                nc.vector.tensor_tensor(
                    Z[:, w : w + 1], Zp[:, 0:1], Zp[:, 1:2], ADD
                )
                nc.vector.tensor_tensor(
                    Z[:, w : w + 1], Z[:, w : w + 1], Zp[:, 2:3], ADD
                )
                nc.vector.reciprocal(Zi[:, w : w + 1], Z[:, w : w + 1])
                for t in range(3):
                    nc.vector.tensor_scalar_mul(
                        cwn[:, T3 * t : T3 * t + T3], cwvs[t][:],
                        Zi[:, w : w + 1],
                    )
                for q in range(Q):
                    tps = psS.tile(
                        [128, 128], F16,
                        tag=("sps0", "sps1", "spst")[q % 3], name="tps",
                    )
                    nc.tensor.transpose(
                        tps[:], cwn[:, 128 * q : 128 * (q + 1)], ident[:]
                    )
                    if w < 2:
                        nc.scalar.copy(cwT[:, q, w, :], tps[:])
                    else:
                        nc.scalar.copy(cwT2[:, q, :], tps[:, 0:64])

        # =========================== flow ==============================
        for it in range(3):
            if it > 0:
                softmax_transpose()
            s_pass(it)
            squash(it)
            if it < 2:
                fill_vs()
                agreement(it)

        nc.sync.dma_start(out_d[:].rearrange("c o b -> o c b"), v_sb[:])

    nc.compile()
    return nc


# =================== host-side prep / entry point =====================

def _prep_shared(W):
    """Per-problem constant tensors (replicated on every core)."""
    W = np.ascontiguousarray(W, np.float32)
    # wfr[rr, q, i, 128g+32j+o] = W[c(g,j), 128q+rr, i, o], pad cols zero
    wv = W.reshape(C, Q, 128, I, O).transpose(2, 1, 3, 0, 4)  # [rr,q,i,c,o]
    wfr = np.zeros((128, Q, I, 10, 32), np.float16)
    for c in range(C):
        wfr[:, :, :, c, :O] = wv[:, :, :, c, :]
    wfr = wfr.reshape(128, Q, I, WF)
    # wte[w, 32j+o, 8r+i] = W[4w+j, r, i, o], dead rows zero
    wt = W.transpose(0, 3, 1, 2).reshape(C, O, RI)  # [c, o, (r i)]
    wte = np.zeros((W3, 4, 32, RI), np.float16)
    for c in range(C):
        w = min(c // 4, 2)
        j = c - 4 * w
        wte[w, j, :O, :] = wt[c]
    wte = wte.reshape(W3, 128, RI)
    H = RI // 2
    wt2 = np.concatenate([wte[2, 0:64, 0:H], wte[2, 0:64, H:RI]], axis=0)
    wt2 = np.ascontiguousarray(wt2)
    ident = np.eye(128, dtype=np.float16)
    return wfr, wte[0:2], wt2, ident


def _prep_core(x_shard):
    """Per-core tensors for one 32-batch shard: xtr and xrep."""
    xs = np.ascontiguousarray(x_shard, np.float32)       # [32, 1152, 8]
    xtr = np.ascontiguousarray(
        xs.reshape(BL, Q, 128, I).transpose(2, 1, 3, 0)
    ).astype(np.float16)                                  # [128, Q, I, 32]
    flat = xs.reshape(BL, RI)                             # [b, 8r+i]
    xrep = np.ascontiguousarray(
        flat[np.arange(128) % BL].astype(np.float16)
    )                                                     # [128, RI]
    return xtr, xrep


_NC_CACHE = {}


def kernel(x, W):
    x = np.asarray(x, np.float32)
    W = np.asarray(W, np.float32)
    if "nc" not in _NC_CACHE:
        _NC_CACHE["nc"] = build_nc()
    nc = _NC_CACHE["nc"]

    wfr, wte, wt2, ident = _prep_shared(W)
    in_maps = []
    for m in range(NC):
        xtr, xrep = _prep_core(x[m * BL : (m + 1) * BL])
        in_maps.append(
            {"xtr": xtr, "wfr": wfr, "wte": wte, "wt2": wt2,
             "xrep": xrep, "ident": ident}
        )

    res = run_bass_kernel_spmd(nc, in_maps, list(range(NC)))
    out = np.empty((C, B, 1, 1, O), np.float32)
    for m in range(NC):
        o = res.results[m]["out"]                         # [C, O, BL]
        out[:, m * BL : (m + 1) * BL, 0, 0, :] = np.asarray(o).transpose(0, 2, 1)
    return out


if __name__ == "__main__":
    d = np.load("/root/problem/ref_data.npz")
    got = kernel(d["x"], d["W"])
    exp = d["expected"]
    err = np.abs(got - exp).max() / np.abs(exp).max()
    print("Relative error:", err)


# revision 23
# speedup vs baseline: 1.4290x; 1.4290x over previous
"""Trainium2 Bass kernel for nn_CapsuleLayer (dynamic routing).

Problem:  u_hat = einsum('bri,crio->cbro', x, W);  3 routing iterations
          (softmax over R, weighted sum, squash, agreement update).
Shapes:   x [256, 1152, 8] f32, W [10, 1152, 8, 16] f32 ->
          out [10, 256, 1, 1, 16] f32.

Strategy (8 NeuronCores, data-parallel over batch, B_loc = 32/core):
  * never materialize u_hat (189 MB) in HBM; all PE data fp16; every
    partition-sliced access sits on a 32-row boundary (walrus rule).
  * s-sums   : per K-block three 128-col stationaries (4+4+2 classes,
    one class per 32-col group: 16 real + 16 pad cols) stream the
    class-blocked y (diagonal blocks used), FWL weight loads; y = cw*x
    built fp16 on DVE/GPSIMD in two q-half stages.
  * agreement: classes stacked into K=128 — per-wave block-diagonal v
    stationary [128,128] (rows 32j+o hold class 4w+j, dead rows zero)
    against an o-major W copy, N=512 streams; ACT drains PSUM->SBUF
    fp16, DVE multiplies by x (2x mode), i-tree l1 on GPSIMD, rest on
    DVE; L fp32.
  * softmax  : ACT exp fp32 (overflow-safe) with accumulated Z, fp16
    normalize, single-pass fp16 PE transposes to r-block partitions.
"""

import sys
from contextlib import ExitStack

import numpy as np

sys.path.insert(0, "/opt/trn_rl_repo")

import concourse.bacc as bacc
import concourse.bass as bass
import concourse.mybir as mybir
import concourse.tile as tile
from concourse.bass_utils import run_bass_kernel_spmd

F32 = mybir.dt.float32
F16 = mybir.dt.float16
MUL = mybir.AluOpType.mult
ADD = mybir.AluOpType.add

B, R, I, C, O = 256, 1152, 8, 10, 16
NC = 8
BL = B // NC          # 32 batch per core
Q = R // 128          # 9 r-blocks of 128
RI = R * I            # 9216
EPS = 1e-7
GCH = 1024            # agreement free-dim chunk (elements of (r,i))
NG = RI // GCH        # 9 chunks
W3 = 3                # (c,b) waves
CB = C * BL           # 320
WF = 320              # wfr columns: 10 class-slots x 32 (16 real + 16 pad)


def build_nc(debug=False):
    nc = bacc.Bacc("TRN2", target_bir_lowering=False, debug=debug)

    xtr_d = nc.declare_dram_parameter("xtr", [128, Q, I, BL], F16, isOutput=False)
    wfr_d = nc.declare_dram_parameter("wfr", [128, Q, I, WF], F16, isOutput=False)
    wte_d = nc.declare_dram_parameter("wte", [2, 128, RI], F16, isOutput=False)
    wt2_d = nc.declare_dram_parameter("wt2", [128, RI // 2], F16, isOutput=False)
    xrep_d = nc.declare_dram_parameter("xrep", [128, RI], F16, isOutput=False)
    ident_d = nc.declare_dram_parameter("ident", [128, 128], F16, isOutput=False)
    out_d = nc.declare_dram_parameter("out", [C, O, BL], F32, isOutput=True)

    with tile.TileContext(nc) as tc, ExitStack() as ctx:
        res = ctx.enter_context(tc.tile_pool(name="res", bufs=1))
        cwp = ctx.enter_context(tc.tile_pool(name="cwp", bufs=1))
        yp = ctx.enter_context(tc.tile_pool(name="yp", bufs=2))
        gsp = ctx.enter_context(tc.tile_pool(name="gsp", bufs=2))
        gmp = ctx.enter_context(tc.tile_pool(name="gmp", bufs=1))
        trp = ctx.enter_context(tc.tile_pool(name="trp", bufs=2))
        smp = ctx.enter_context(tc.tile_pool(name="smp", bufs=1))
        psS = ctx.enter_context(
            tc.tile_pool(name="psS", bufs=1, space=bass.MemorySpace.PSUM)
        )
        psG = ctx.enter_context(
            tc.tile_pool(name="psG", bufs=2, space=bass.MemorySpace.PSUM)
        )
        psT = ctx.enter_context(
            tc.tile_pool(name="psT", bufs=1, space=bass.MemorySpace.PSUM)
        )

        # ---- resident tensors -------------------------------------
        xtr = res.tile([128, Q, I, BL], F16)
        wfr = res.tile([128, Q, I, WF], F16)
        wte = res.tile([128, 2, RI], F16)    # rows 32j+o = W[4w+j]
        wt2 = res.tile([128, RI // 2], F16)  # wave-2 plane folded in half
        xrep = res.tile([128, RI], F16)
        ident = res.tile([128, 128], F16)
        L = res.tile([128, W3, R], F16)
        cwT = res.tile([128, Q, 2, 128], F16)
        cwT2 = res.tile([128, Q, 64], F16)
        Z = res.tile([128, W3], F32)
        Zi = res.tile([128, W3], F32)
        vsE = res.tile([128, W3, 128], F16)  # block-diag v per wave
        ones16 = res.tile([16, 16], F32)
        sps_sb = res.tile([16, C, BL], F32)  # extracted s, [o, c, b]
        v_sb = res.tile([16, C, BL], F32)    # squash output, [o, c, b]

        nc.sync.dma_start(xtr[:], xtr_d[:])
        for q in range(3):
            nc.sync.dma_start(wfr[:, q], wfr_d[:, q])
        nc.sync.dma_start(wte[:, 0, :], wte_d[0])
        for q in range(3, Q):
            nc.sync.dma_start(wfr[:, q], wfr_d[:, q])
        nc.sync.dma_start(xrep[:], xrep_d[:])
        nc.sync.dma_start(wte[:, 1, :], wte_d[1])
        nc.sync.dma_start(wt2[:], wt2_d[:])
        nc.sync.dma_start(ident[:], ident_d[:])
        nc.vector.memset(ones16[:], 1.0)
        nc.vector.memset(vsE[:], 0.0)

        # class c -> (s_pass group g, 32-row/col slot j)
        def slot(c):
            return (c // 4, c % 4) if c < 8 else (2, c - 8)

        # ---------------------------------------------------------------
        def build_y(c, qlo, qhi, yall):
            """y[rr, q, i, 128g+32j+b] = cw[c,b,128q+rr] * x[b,128q+rr,i]."""
            w = min(c // 4, 2)
            k = c - 4 * w
            g, j = slot(c)
            nq = qhi - qlo
            cw_blk = (
                cwT[:, qlo:qhi, w, 32 * k : 32 * k + 32]
                if w < 2
                else cwT2[:, qlo:qhi, 32 * k : 32 * k + 32]
            )
            cw_src = cw_blk.unsqueeze(2).broadcast_to([128, nq, I, BL])
            col = 128 * g + 32 * j
            eng = nc.gpsimd if c in (3, 6, 9) else nc.vector
            eng.tensor_tensor(
                yall[:, 0 : qhi - qlo, :, col : col + 32],
                xtr[:, qlo:qhi],
                cw_src,
                MUL,
            )

        def s_pass(it):
            """s[o,c,b] = sum_{r,i} rhs_c[r,i,b] * W[c,r,i,o].

            Three stationaries per K-block (classes 0-3 / 4-7 / 8-9, one
            class per 32-col group, pad cols zero).  it==0 streams xtr
            (shared rhs); it>0 streams the class-blocked y (diagonal
            class blocks used)."""
            ps = [
                psS.tile([128, 128 if it else BL], F32, tag="sps0", name="sps0"),
                psS.tile([128, 128 if it else BL], F32, tag="sps1", name="sps1"),
                psS.tile([128, 64 if it else BL], F32, tag="spst", name="spst"),
            ]
            nn = [128, 128, 64] if it else [BL, BL, BL]
            g = 0
            for qlo, qhi in ((0, 5), (5, 9)):
                yall = None
                if it:
                    yall = yp.tile(
                        [128, 5, I, 320], F16, tag="yall", name="yall"
                    )
                    for c in range(C):
                        build_y(c, qlo, qhi, yall)
                for q in range(qlo, qhi):
                    for i in range(I):
                        for grp in range(3):
                            if it:
                                base = 128 * grp
                                rhs = yall[:, q - qlo, i, base : base + nn[grp]]
                            else:
                                rhs = xtr[:, q, i, :]
                            m = 128 if grp < 2 else 64
                            nc.tensor.matmul(
                                ps[grp][0:m, :],
                                wfr[:, q, i, 128 * grp : 128 * grp + m],
                                rhs,
                                start=(g == 0),
                                stop=(g == Q * I - 1),
                            )
                        g += 1
            # extract per-class [16,32] diagonal blocks into [o, c, b]
            for c in range(C):
                grp, j = slot(c)
                cols = slice(32 * j, 32 * j + 32) if it else slice(0, BL)
                nc.vector.tensor_copy(
                    sps_sb[:, c, :], ps[grp][32 * j : 32 * j + 16, cols]
                )

        # ---------------------------------------------------------------
        def squash(it):
            """v_sb = squash(s) over o;  it==0 folds the uniform 1/R weight.

            Scalars kept [16, C*BL] (replicated rows via the ones16 matmul)
            so no partition broadcast is needed."""
            sq = smp.tile([16, C * BL], F32, tag="sq")
            nc.scalar.activation(
                sq[:],
                sps_sb[:].rearrange("o c b -> o (c b)"),
                mybir.ActivationFunctionType.Square,
            )
            snps = psS.tile([16, C * BL], F32, tag="spst")
            nc.tensor.matmul(snps[:], ones16[:], sq[:], start=True, stop=True)
            sn = smp.tile([16, C * BL], F32, tag="sn")
            if it == 0:
                nc.vector.tensor_scalar_mul(sn[:], snps[:], 1.0 / (R * R))
            else:
                nc.vector.tensor_copy(sn[:], snps[:])
            u1 = smp.tile([16, C * BL], F32, tag="u1")
            u2 = smp.tile([16, C * BL], F32, tag="u2")
            u3 = smp.tile([16, C * BL], F32, tag="sq")   # sq slot is free now
            u4 = smp.tile([16, C * BL], F32, tag="u1")   # u1 free after sqrt
            ub = smp.tile([16, C * BL], F32, tag="u2")   # u2 free after u4
            f = smp.tile([16, C * BL], F32, tag="sq")    # u3 free after u4
            nc.vector.tensor_scalar_add(u1[:], sn[:], EPS)
            nc.scalar.activation(u2[:], u1[:], mybir.ActivationFunctionType.Sqrt)
            nc.vector.tensor_scalar_add(u3[:], sn[:], 1.0)
            nc.vector.tensor_tensor(u4[:], u2[:], u3[:], MUL)
            nc.vector.reciprocal(ub[:], u4[:])
            nc.vector.tensor_tensor(f[:], sn[:], ub[:], MUL)
            if it == 0:
                nc.vector.tensor_scalar_mul(f[:], f[:], 1.0 / R)
            nc.vector.tensor_tensor(
                v_sb[:].rearrange("o c b -> o (c b)"),
                sps_sb[:].rearrange("o c b -> o (c b)"),
                f[:],
                MUL,
            )

        def fill_vs():
            """Stage v into the per-wave block-diagonal stationaries."""
            for c in range(C):
                w = min(c // 4, 2)
                j = c - 4 * w
                nc.vector.tensor_copy(
                    vsE[32 * j : 32 * j + 16, w, 32 * j : 32 * j + 32],
                    v_sb[:, c, :],
                )
                if c >= 8:  # duplicate for the folded wave-2 W plane
                    nc.vector.tensor_copy(
                        vsE[64 + 32 * j : 64 + 32 * j + 16, w,
                            32 * j : 32 * j + 32],
                        v_sb[:, c, :],
                    )

        # ---------------------------------------------------------------
        def agreement(it):
            """L[p, w, r] (+)= sum_o v[c,b,o]*W[c,r,i,o] (*) x[b,r,i], sum_i.

            One K=128 matmul per 512-chunk per wave (block-diagonal v
            against o-major W); chunk pipeline: PE -> ACT drain fp16 ->
            DVE multiply -> GPSIMD l1 -> DVE l2/l3 -> L fp32."""
            nr = GCH // I  # 128 r per chunk
            for n0 in range(NG):
                off = n0 * GCH
                r0 = off // I
                # gm[p, w, (r i)] for all three waves of this chunk
                gm = gmp.tile([128, W3, GCH], F16, tag="gm")
                H = RI // 2
                for w in range(W3):
                    npart = 128 if w < 2 else 64
                    gps = psG.tile([128, GCH], F32, tag="gps")
                    if w < 2:
                        for sub in range(0, GCH, 512):
                            nc.tensor.matmul(
                                gps[:, sub : sub + 512],
                                vsE[:, w, :],
                                wte[:, w, off + sub : off + sub + 512],
                                start=True,
                                stop=True,
                            )
                    else:
                        # folded plane: pick the 64-row half per (ri) range
                        o0 = off
                        while o0 < off + GCH:
                            half = o0 // H
                            o1 = min(off + GCH, (half + 1) * H, o0 + 512)
                            nc.tensor.matmul(
                                gps[:, o0 - off : o1 - off],
                                vsE[64 * half : 64 * half + 64, 2, :],
                                wt2[64 * half : 64 * half + 64,
                                    o0 - half * H : o1 - half * H],
                                start=True,
                                stop=True,
                            )
                            o0 = o1
                    # ACT drain PSUM -> SBUF fp16 (frees PSUM, DVE gets 2x)
                    gs = gsp.tile([128, GCH], F16, tag="gs")
                    nc.scalar.copy(gs[:npart, :], gps[:npart, :])
                    # gm_w = gs * xrep  (fp16, contiguous); w2 on GPSIMD
                    eng = nc.vector if w < 2 else nc.gpsimd
                    eng.tensor_tensor(
                        gm[:npart, w, :],
                        gs[:npart, :],
                        xrep[:npart, off : off + GCH],
                        MUL,
                    )
                # i-tree batched over waves (8 -> 4 -> 2 -> L);
                # l2/l3 run in place inside l1's tile
                l1 = trp.tile([128, W3, GCH // 2], F16, tag="l1")
                gmv = gm.rearrange("p w (r i) -> p w r i", i=I)
                l1v = l1.rearrange("p w (r i) -> p w r i", i=4)
                nc.vector.tensor_tensor(
                    l1v[:], gmv[:, :, :, 0:4], gmv[:, :, :, 4:8], ADD
                )
                nc.gpsimd.tensor_tensor(
                    l1v[:, :, :, 0:2], l1v[:, :, :, 0:2], l1v[:, :, :, 2:4],
                    ADD,
                )
                if it == 0:
                    nc.vector.tensor_tensor(
                        L[:, :, r0 : r0 + nr],
                        l1v[:, :, :, 0],
                        l1v[:, :, :, 1],
                        ADD,
                    )
                else:
                    nc.gpsimd.tensor_tensor(
                        l1v[:, :, :, 0], l1v[:, :, :, 0], l1v[:, :, :, 1], ADD
                    )
                    nc.vector.tensor_tensor(
                        L[:, :, r0 : r0 + nr],
                        L[:, :, r0 : r0 + nr],
                        l1v[:, :, :, 0],
                        ADD,
                    )

        # ---------------------------------------------------------------
        def softmax_transpose():
            """cw = softmax_r(L); exp fp32 (overflow-safe), fp16 normalize,
            single-pass fp16 transposes into cwT."""
            T3 = R // 3
            for w in range(W3):
                cwn = cwp.tile([128, R], F16, tag="cwn")
                Zp = cwp.tile([128, 3], F32, tag="Zp")
                cwvs = []
                for t in range(3):
                    cwv = cwp.tile([128, T3], F32, tag=f"cwv{t}", name="cwv")
                    nc.scalar.activation(
                        cwv[:],
                        L[:, w, T3 * t : T3 * t + T3],
                        mybir.ActivationFunctionType.Exp,
                        accum_out=Zp[:, t : t + 1],
                    )
                    cwvs.append(cwv)
                nc.vector.tensor_tensor(
                    Z[:, w : w + 1], Zp[:, 0:1], Zp[:, 1:2], ADD
                )
                nc.vector.tensor_tensor(
                    Z[:, w : w + 1], Z[:, w : w + 1], Zp[:, 2:3], ADD
                )
                nc.vector.reciprocal(Zi[:, w : w + 1], Z[:, w : w + 1])
                for t in range(3):
                    nc.vector.tensor_scalar_mul(
                        cwn[:, T3 * t : T3 * t + T3], cwvs[t][:],
                        Zi[:, w : w + 1],
                    )
                for q in range(Q):
                    tps = psT.tile([128, 128], F16, tag="tps")
                    nc.tensor.transpose(
                        tps[:], cwn[:, 128 * q : 128 * (q + 1)], ident[:]
                    )
                    if w < 2:
                        nc.scalar.copy(cwT[:, q, w, :], tps[:])
                    else:
                        nc.scalar.copy(cwT2[:, q, :], tps[:, 0:64])

        # =========================== flow ==============================
        for it in range(3):
            if it > 0:
                softmax_transpose()
            s_pass(it)
            squash(it)
            if it < 2:
                fill_vs()
                agreement(it)

        nc.sync.dma_start(out_d[:].rearrange("c o b -> o c b"), v_sb[:])

    nc.compile()
    return nc


# =================== host-side prep / entry point =====================

def _prep_shared(W):
    """Per-problem constant tensors (replicated on every core)."""
    W = np.ascontiguousarray(W, np.float32)
    # wfr[rr, q, i, 128g+32j+o] = W[c(g,j), 128q+rr, i, o], pad cols zero
    wv = W.reshape(C, Q, 128, I, O).transpose(2, 1, 3, 0, 4)  # [rr,q,i,c,o]
    wfr = np.zeros((128, Q, I, 10, 32), np.float16)
    for c in range(C):
        wfr[:, :, :, c, :O] = wv[:, :, :, c, :]
    wfr = wfr.reshape(128, Q, I, WF)
    # wte[w, 32j+o, 8r+i] = W[4w+j, r, i, o], dead rows zero
    wt = W.transpose(0, 3, 1, 2).reshape(C, O, RI)  # [c, o, (r i)]
    wte = np.zeros((W3, 4, 32, RI), np.float16)
    for c in range(C):
        w = min(c // 4, 2)
        j = c - 4 * w
        wte[w, j, :O, :] = wt[c]
    wte = wte.reshape(W3, 128, RI)
    H = RI // 2
    wt2 = np.concatenate([wte[2, 0:64, 0:H], wte[2, 0:64, H:RI]], axis=0)
    wt2 = np.ascontiguousarray(wt2)
    ident = np.eye(128, dtype=np.float16)
    return wfr, wte[0:2], wt2, ident


def _prep_core(x_shard):
    """Per-core tensors for one 32-batch shard: xtr and xrep."""
    xs = np.ascontiguousarray(x_shard, np.float32)       # [32, 1152, 8]
    xtr = np.ascontiguousarray(
        xs.reshape(BL, Q, 128, I).transpose(2, 1, 3, 0)
    ).astype(np.float16)                                  # [128, Q, I, 32]
    flat = xs.reshape(BL, RI)                             # [b, 8r+i]
    xrep = np.ascontiguousarray(
        flat[np.arange(128) % BL].astype(np.float16)
    )                                                     # [128, RI]
    return xtr, xrep


_NC_CACHE = {}


def kernel(x, W):
    x = np.asarray(x, np.float32)
    W = np.asarray(W, np.float32)
    if "nc" not in _NC_CACHE:
        _NC_CACHE["nc"] = build_nc()
    nc = _NC_CACHE["nc"]

    wfr, wte, wt2, ident = _prep_shared(W)
    in_maps = []
    for m in range(NC):
        xtr, xrep = _prep_core(x[m * BL : (m + 1) * BL])
        in_maps.append(
            {"xtr": xtr, "wfr": wfr, "wte": wte, "wt2": wt2,
             "xrep": xrep, "ident": ident}
        )

    res = run_bass_kernel_spmd(nc, in_maps, list(range(NC)))
    out = np.empty((C, B, 1, 1, O), np.float32)
    for m in range(NC):
        o = res.results[m]["out"]                         # [C, O, BL]
        out[:, m * BL : (m + 1) * BL, 0, 0, :] = np.asarray(o).transpose(0, 2, 1)
    return out


if __name__ == "__main__":
    d = np.load("/root/problem/ref_data.npz")
    got = kernel(d["x"], d["W"])
    exp = d["expected"]
    err = np.abs(got - exp).max() / np.abs(exp).max()
    print("Relative error:", err)
